# revision 7
# baseline (speedup 1.0000x reference)
"""Trainium2 Bass kernel for nn_Attention_52536039965434 (v3).

Reference computation (B=2, SQ=SK=2048, H=1024, NH=16, HD=64):
    qkv = x @ c_attn_w + b ; per-head attention with multiplicative mask
    (post-score, pre-softmax); attn @ c_proj_w + b; gelu(cat(x, attn) @ mlp_w + b)

Sharding (8 cores): core c -> (b = c//4, g = c%4). Data parallel over batch;
attention is tensor parallel over 4 head-groups (4 heads = 256 dims each);
the MLP tail is Q-SHARDED: core g produces the FULL 1024 output features for
its q-quarter [g*512, (g+1)*512).  kernel() reassembles in python, so the
only collective is ONE ReduceScatter per batch group: the 4 cores' partial
c_proj outputs (each contracted over its local 256 attn features, full 1024
outs, all 2048 q) are summed and scattered by q-quarter.

Engine plan:
  - K^T/V phase first (PE dense, ACT idle), then per q-block: QT(qb) +
    attention(qb) + partial c_proj(qb), with mlp1 (full-width, local
    q-quarter, zero dependencies) emitted as PE filler between blocks.  The
    Tile list-scheduler fills PE exp-stalls with these matmuls, keeping the
    PE HAM-warm.
  - Q/K biases folded into the PSUM evacuation (DVE ops with per-partition
    bias APs); c_proj bias folded host-side into the gelu bias
    (cpb @ mlp_w2 + mlp_b); V bias via an augmented-row matmul.
  - exp instructions are [128, 1024] (2 PSUM banks, double-buffered); the
    softmax denominator rides as the 65th V column.
Attention matmuls are float32r; the MLP tail runs bf16 (weights + acts) to
fit SBUF and halve the collective bytes.
"""

import os

import numpy as np
import ml_dtypes

import concourse.bacc as bacc
import concourse.mybir as mybir
import concourse.tile as tile
from concourse import bass_utils

# ---- problem dims (hardcoded per contest contract) ----
B = 2
S = 2048          # SQ == SK
H = 1024
NH = 16
HD = 64
NCORES = 8
TP = 4            # cores per batch (head groups / q-quarters)
HPC = NH // TP    # heads per core = 4
DH = HPC * HD     # per-core head width = 256
QB = 512          # q-block (matmul moving free dim)
P = 128

F32 = mybir.dt.float32
F32R = mybir.dt.float32r
BF16 = mybir.dt.bfloat16
AF = mybir.ActivationFunctionType
ALU = mybir.AluOpType
NPBF16 = ml_dtypes.bfloat16


def _build_nc(reps=1):
    """Build + compile the single SPMD Bass program (same NEFF on all 8 cores)."""
    gelu_mode = os.environ.get("KERNEL_GELU", "builtin")
    s = S
    nq = s // QB          # q blocks = 4
    nkt = s // P          # k tiles = 16
    nf = H // P           # feature tiles of H = 8

    nc = bacc.Bacc(
        "TRN2", target_bir_lowering=False, debug=False, num_devices=NCORES
    )

    # ---- kernel I/O (per-core contents supplied via in_maps) ----
    xatt = nc.dram_tensor("xatt", [H, s], F32R, kind="ExternalInput").ap()
    xatd = nc.dram_tensor("xatd", [H + 1, s], F32R, kind="ExternalInput").ap()
    xmlp_d = nc.dram_tensor("xmlp", [H, QB], BF16, kind="ExternalInput").ap()
    wq_d = nc.dram_tensor("wq", [H, DH], F32R, kind="ExternalInput").ap()
    wk_d = nc.dram_tensor("wk", [H, DH], F32R, kind="ExternalInput").ap()
    wv_d = nc.dram_tensor("wv", [H + 1, DH], F32R, kind="ExternalInput").ap()
    qkb_d = nc.dram_tensor("qkbias", [P, 4], F32, kind="ExternalInput").ap()
    beff_d = nc.dram_tensor("beff", [P, nf], F32, kind="ExternalInput").ap()
    mask_d = nc.dram_tensor("maskrep", [P, s], F32, kind="ExternalInput").ap()
    cpw_d = nc.dram_tensor("cprojw", [DH, H], F32R, kind="ExternalInput").ap()
    mw1_d = nc.dram_tensor("mlpw1", [H, H], BF16, kind="ExternalInput").ap()
    mw2_d = nc.dram_tensor("mlpw2", [H, H], BF16, kind="ExternalInput").ap()
    outQ = nc.dram_tensor("outQ", [H, QB], F32, kind="ExternalOutput").ap()

    rg = [[0, 1, 2, 3], [4, 5, 6, 7]]

    with tile.TileContext(nc) as tc:
      for rep in range(reps):
        with (
            tc.tile_pool(name=f"dram{rep}", bufs=1, space="DRAM") as dram,
            tc.tile_pool(name=f"w{rep}", bufs=1) as wpool,
            tc.tile_pool(name=f"per{rep}", bufs=1) as per,
            tc.tile_pool(name=f"xstream{rep}", bufs=4) as xstream,
            tc.tile_pool(name=f"augstream{rep}", bufs=2) as augstream,
            tc.tile_pool(name=f"qt{rep}", bufs=2) as qtpool,
            tc.tile_pool(name=f"at{rep}", bufs=2) as atpool,
            tc.tile_pool(name=f"e{rep}", bufs=2) as epool,
            tc.tile_pool(name=f"zp{rep}", bufs=3) as zppool,
            tc.tile_pool(name=f"go{rep}", bufs=2) as gopool,
            tc.tile_pool(name=f"small{rep}", bufs=2) as small,
            tc.tile_pool(name=f"psg{rep}", bufs=2, space="PSUM") as psg,
            tc.tile_pool(name=f"pss{rep}", bufs=2, space="PSUM") as pss,
            tc.tile_pool(name=f"psv{rep}", bufs=2, space="PSUM") as psv,
        ):
            # internal DRAM for the ReduceScatter: rows qb*1024 + ot*128,
            # so RS chunk r (rows [r*1024,(r+1)*1024)) is q-block r's full z.
            zin = dram.tile([nq * H, QB], BF16, tag="zin", name=f"zin{rep}")
            zloc = dram.tile([H, QB], BF16, tag="zloc", name=f"zloc{rep}")

            # ---------- weight / bias / mask loads ----------
            wq_sb = wpool.tile([P, nf * DH], F32R, tag="wq")
            wk_sb = wpool.tile([P, nf * DH], F32R, tag="wk")
            wv_sb = wpool.tile([P, nf * DH], F32R, tag="wv")
            wvb_sb = wpool.tile([1, DH], F32R, tag="wvb")
            qkb_sb = wpool.tile([P, 4], F32, tag="qkb")
            beff_sb = wpool.tile([P, nf], F32, tag="beff")
            mask_sb = wpool.tile([P, s], F32, tag="mask")
            cproj_sb = wpool.tile([P, 2 * H], F32R, tag="cproj")
            mw1_sb = wpool.tile([P, nf * H], BF16, tag="mw1")
            mw2_sb = wpool.tile([P, nf * H], BF16, tag="mw2")
            xmlp_sb = wpool.tile([P, nf * QB], BF16, tag="xmlp")
            for w_d, w_sb in ((wq_d, wq_sb), (wk_d, wk_sb), (wv_d, wv_sb)):
                nc.sync.dma_start(
                    out=w_sb[:].rearrange("p (t d) -> p t d", d=DH),
                    in_=w_d[:H].rearrange("(t p) d -> p t d", p=P),
                )
            nc.sync.dma_start(out=wvb_sb[:], in_=wv_d[H : H + 1])
            nc.sync.dma_start(out=qkb_sb[:], in_=qkb_d[:])
            nc.sync.dma_start(out=beff_sb[:], in_=beff_d[:])
            nc.sync.dma_start(out=mask_sb[:], in_=mask_d[:])
            nc.sync.dma_start(
                out=cproj_sb[:].rearrange("p (c o) -> p c o", o=H),
                in_=cpw_d[:].rearrange("(c p) o -> p c o", p=P),
            )
            for w_d, w_sb in ((mw1_d, mw1_sb), (mw2_d, mw2_sb)):
                nc.sync.dma_start(
                    out=w_sb[:].rearrange("p (t o) -> p t o", o=H),
                    in_=w_d[:].rearrange("(t p) o -> p t o", p=P),
                )
            nc.sync.dma_start(
                out=xmlp_sb[:].rearrange("p (t q) -> p t q", q=QB),
                in_=xmlp_d[:].rearrange("(t p) q -> p t q", p=P),
            )

            # ---------- persistent activations ----------
            KT_sb = per.tile([P, 2 * s], F32R, tag="kt")    # pair p at cols p*s
            V_sb = per.tile([P, nkt * 260], F32R, tag="v")  # per kt: 4 heads x 65
            out1_sb = per.tile([P, nf * QB], BF16, tag="out1")  # ot*QB + q

            # ones columns of the augmented V (denominator trick)
            for kt in range(nkt):
                nc.vector.memset(
                    V_sb[:, kt * 260 : (kt + 1) * 260]
                    .rearrange("p (h c) -> p h c", c=65)[:, :, 64:65]
                    .opt()
                    .bitcast(F32),
                    1.0,
                )

            # mlp1 emission units: full 1024 outs for the local q-quarter.
            # Zero data dependencies -> perfect PE filler between phases.
            def emit_mlp1(ot):
                ps = psg.tile([P, QB], F32, tag="g", name=f"m1ps{ot}")
                for t in range(nf):
                    nc.tensor.matmul(
                        ps[:],
                        lhsT=mw1_sb[:, t * H + ot * P : t * H + (ot + 1) * P],
                        rhs=xmlp_sb[:, t * QB : (t + 1) * QB],
                        start=(t == 0),
                        stop=(t == nf - 1),
                    )
                nc.vector.tensor_copy(
                    out1_sb[:, ot * QB : (ot + 1) * QB], ps[:]
                )

            # ---------- phase KV: K^T (bias+mask fused) and V (k-major) ----
            for kb in range(nq):
                cs = slice(kb * QB, (kb + 1) * QB)
                x_ch = [
                    xstream.tile([P, (nf // 2) * QB], F32R, tag="xch",
                                 name=f"xd{kb}_{i}")
                    for i in range(2)
                ]
                x_aug = augstream.tile([1, QB], F32R, tag="xaug")
                for i in range(2):
                    nc.sync.dma_start(
                        out=x_ch[i][:].rearrange("p (t q) -> p t q", q=QB),
                        in_=xatd[i * (H // 2) : (i + 1) * (H // 2)]
                        .rearrange("(t p) q -> p t q", p=P)[:, :, cs],
                    )
                nc.sync.dma_start(out=x_aug[:], in_=xatd[H : H + 1, cs])
                for p in range(2):
                    ps = psg.tile([P, QB], F32, tag="g", name=f"kps{kb}_{p}")
                    for t in range(nf):
                        nc.tensor.matmul(
                            ps[:],
                            lhsT=w_slice(wk_sb, t, p),
                            rhs=x_ch[t // 4][:, (t % 4) * QB : (t % 4 + 1) * QB],
                            start=(t == 0),
                            stop=(t == nf - 1),
                        )
                    # evacuation: (k + bias) * mask in one DVE op
                    nc.vector.scalar_tensor_tensor(
                        KT_sb[:, p * s + kb * QB : p * s + (kb + 1) * QB],
                        ps[:],
                        qkb_sb[:, 2 + p : 3 + p],
                        mask_sb[:, cs],
                        ALU.add,
                        ALU.mult,
                    )
                for sub in range(QB // P):
                    kt = kb * (QB // P) + sub
                    pv_ps = psg.tile([P, QB], F32, tag="g", name=f"vps{kb}_{sub}")
                    for t in range(nf):
                        nc.tensor.matmul(
                            pv_ps[:, :DH],
                            lhsT=x_ch[t // 4][
                                :,
                                (t % 4) * QB + sub * P : (t % 4) * QB
                                + (sub + 1) * P,
                            ],
                            rhs=wv_sb[:, t * DH : (t + 1) * DH],
                            start=(t == 0),
                            stop=False,
                        )
                    nc.tensor.matmul(
                        pv_ps[:, :DH],
                        lhsT=x_aug[0:1, sub * P : (sub + 1) * P],
                        rhs=wvb_sb[:],
                        start=False,
                        stop=True,
                    )
                    nc.vector.tensor_copy(
                        V_sb[:, kt * 260 : (kt + 1) * 260]
                        .rearrange("p (h c) -> p h c", c=65)[:, :, 0:64],
                        pv_ps[:, :DH].rearrange("p (h c) -> p h c", c=HD),
                    )

            # ---------- main loop: QT + attention + c_proj (+ mlp1 filler) --
            for qb in range(nq):
                cs = slice(qb * QB, (qb + 1) * QB)
                x_ch = [
                    xstream.tile([P, (nf // 2) * QB], F32R, tag="xch",
                                 name=f"xq{qb}_{i}")
                    for i in range(2)
                ]
                for i in range(2):
                    nc.sync.dma_start(
                        out=x_ch[i][:].rearrange("p (t q) -> p t q", q=QB),
                        in_=xatt[i * (H // 2) : (i + 1) * (H // 2)]
                        .rearrange("(t p) q -> p t q", p=P)[:, :, cs],
                    )
                # QT(qb): 2 output halves, bias fused into evacuation
                QT_t = qtpool.tile([P, 2 * QB], F32R, tag="qt")
                for p in range(2):
                    ps = psg.tile([P, QB], F32, tag="g", name=f"qps{qb}_{p}")
                    for t in range(nf):
                        nc.tensor.matmul(
                            ps[:],
                            lhsT=w_slice(wq_sb, t, p),
                            rhs=x_ch[t // 4][:, (t % 4) * QB : (t % 4 + 1) * QB],
                            start=(t == 0),
                            stop=(t == nf - 1),
                        )
                    nc.vector.tensor_scalar(
                        QT_t[:, p * QB : (p + 1) * QB],
                        ps[:],
                        qkb_sb[:, p : p + 1],
                        None,
                        ALU.add,
                    )
                # mlp1 filler (2 of 8 units per qb)
                emit_mlp1(2 * qb)
                emit_mlp1(2 * qb + 1)

                # ---- attention(qb) ----
                attnT_t = atpool.tile([P, 2 * QB], F32R, tag="at")
                for p in range(2):
                    pvs = [
                        psv.tile([65, QB], F32, tag="pv", name=f"pv{qb}_{p}_{h}")
                        for h in range(2)
                    ]
                    sc_tiles = {}

                    def emit_scores(kt, p=p, qb=qb, sc_tiles=sc_tiles):
                        sc = pss.tile([P, 2 * QB], F32, tag="sc",
                                      name=f"sc{qb}_{p}_{kt}")
                        sc_tiles[kt] = sc
                        for half in range(2):
                            nc.tensor.matmul(
                                sc[:, half * QB : (half + 1) * QB],
                                lhsT=KT_sb[
                                    64 * half : 64 * half + 64,
                                    p * s + kt * P : p * s + (kt + 1) * P,
                                ],
                                rhs=QT_t[64 * half : 64 * half + 64,
                                         p * QB : (p + 1) * QB],
                                start=True,
                                stop=True,
                                tile_position=(64 * half, 0),
                            )

                    emit_scores(0)
                    for kt in range(nkt):
                        if kt + 1 < nkt:
                            emit_scores(kt + 1)
                        e = epool.tile([P, 2 * QB], F32R, tag="e")
                        nc.scalar.activation(e[:], sc_tiles.pop(kt)[:], AF.Exp)
                        for half in range(2):
                            h = 2 * p + half
                            nc.tensor.matmul(
                                pvs[half][:],
                                lhsT=V_sb[:, kt * 260 + h * 65 : kt * 260
                                          + (h + 1) * 65],
                                rhs=e[:, half * QB : (half + 1) * QB],
                                start=(kt == 0),
                                stop=(kt == nkt - 1),
                            )
                    # normalize by the denominator (row 64), store attn^T
                    for half in range(2):
                        rec = small.tile([1, QB], F32, tag="rec")
                        nc.vector.reciprocal(rec[:], pvs[half][64:65, :])
                        recb = small.tile([64, QB], F32, tag="recb")
                        nc.gpsimd.partition_broadcast(recb[:], rec[:], channels=64)
                        nc.vector.tensor_tensor(
                            attnT_t[64 * half : 64 * half + 64,
                                    p * QB : (p + 1) * QB],
                            pvs[half][0:64, :],
                            recb[:],
                            ALU.mult,
                        )

                # ---- c_proj partials (full 1024 outs, contract local 256) ----
                for ot in range(nf):
                    ps = psg.tile([P, QB], F32, tag="g", name=f"cps{qb}_{ot}")
                    for fc in range(2):
                        nc.tensor.matmul(
                            ps[:],
                            lhsT=cproj_sb[:, fc * H + ot * P : fc * H
                                          + (ot + 1) * P],
                            rhs=attnT_t[:, fc * QB : (fc + 1) * QB],
                            start=(fc == 0),
                            stop=(fc == 1),
                        )
                    zp = zppool.tile([P, QB], BF16, tag="zp")
                    nc.vector.tensor_copy(zp[:], ps[:])
                    nc.sync.dma_start(
                        out=zin[qb * H + ot * P : qb * H + (ot + 1) * P, :],
                        in_=zp[:],
                    )

            # ---------- ReduceScatter: sum f-shard partials, scatter by q ---
            nc.gpsimd.collective_compute(
                "ReduceScatter", ALU.add, replica_groups=rg,
                ins=[zin[:].opt()], outs=[zloc[:].opt()],
            )

            # ---------- mlp2 (full 1024 outs, local q-quarter) ----------
            z_sb = per.tile([P, nf * QB], BF16, tag="zs")
            nc.sync.dma_start(
                out=z_sb[:].rearrange("p (t q) -> p t q", q=QB),
                in_=zloc[:].rearrange("(t p) q -> p t q", p=P),
            )
            for ot in range(nf):
                ps = psg.tile([P, QB], F32, tag="g", name=f"m2ps{ot}")
                for t in range(nf):
                    nc.tensor.matmul(
                        ps[:],
                        lhsT=mw2_sb[:, t * H + ot * P : t * H + (ot + 1) * P],
                        rhs=z_sb[:, t * QB : (t + 1) * QB],
                        start=(t == 0),
                        stop=(t == nf - 1),
                    )
                o1 = out1_sb[:, ot * QB : (ot + 1) * QB]
                nc.vector.tensor_add(o1, ps[:], o1)

                # ---------- gelu (+ folded mlp/cproj bias) ----------
                gout = gopool.tile([P, QB], F32, tag="gout")
                if gelu_mode == "builtin":
                    nc.scalar.activation(
                        gout[:], o1, AF.Gelu_apprx_tanh,
                        bias=beff_sb[:, ot : ot + 1],
                    )
                else:
                    # exact GPT-2 tanh gelu from primitives (Tanh shares the
                    # exp table set -> no ACT table switch)
                    tmp = gopool.tile([P, QB], F32, tag="gtmp")
                    u = gopool.tile([P, QB], F32, tag="gu")
                    th = gopool.tile([P, QB], F32, tag="gth")
                    nc.vector.tensor_scalar(
                        tmp[:], o1, beff_sb[:, ot : ot + 1], None, ALU.add
                    )
                    nc.vector.tensor_mul(u[:], tmp[:], tmp[:])
                    nc.vector.tensor_mul(u[:], u[:], tmp[:])
                    nc.vector.scalar_tensor_tensor(
                        u[:], u[:], 0.044715, tmp[:], ALU.mult, ALU.add
                    )
                    nc.scalar.activation(
                        th[:], u[:], AF.Tanh, scale=0.7978845608028654
                    )
                    nc.vector.scalar_tensor_tensor(
                        th[:], th[:], 1.0, tmp[:], ALU.add, ALU.mult
                    )
                    nc.vector.tensor_scalar_mul(gout[:], th[:], 0.5)
                nc.sync.dma_start(
                    out=outQ[ot * P : (ot + 1) * P, :],
                    in_=gout[:],
                )

    nc.compile()
    return nc


def w_slice(w_sb, t, p):
    """lhsT [128, 128] slice: f-tile t, output half p, of a [128, nt*256] layout."""
    return w_sb[:, t * DH + p * P : t * DH + (p + 1) * P]


_NC_CACHE = {}
LAST_RESULTS = None


def _get_nc():
    return _get_nc_reps(1)


def _get_nc_reps(reps):
    key = ("reps", reps)
    if key not in _NC_CACHE:
        _NC_CACHE[key] = _build_nc(reps=reps)
    return _NC_CACHE[key]


def kernel(**inputs):
    global LAST_RESULTS
    nc = _get_nc()
    in_maps = make_in_maps(inputs)

    trace = bool(int(os.environ.get("KERNEL_TRACE", "0")))
    res = bass_utils.run_bass_kernel_spmd(
        nc, in_maps, core_ids=list(range(NCORES)), trace=trace
    )
    LAST_RESULTS = res

    out = np.empty((B, S, H), np.float32)
    for c in range(NCORES):
        b, g = c // TP, c % TP
        out[b, g * QB : (g + 1) * QB, :] = res.results[c]["outQ"].T
    return out


def make_in_maps(inputs):
    xq = np.ascontiguousarray(np.asarray(inputs["attender_seq"], np.float32))
    xk = np.ascontiguousarray(np.asarray(inputs["attendee_seq"], np.float32))
    mask = np.asarray(inputs["attendee_mask"]).astype(np.float32)
    caw = np.asarray(inputs["c_attn_w"], np.float32)
    cab = np.asarray(inputs["c_attn_b"], np.float32)
    cpw = np.ascontiguousarray(np.asarray(inputs["c_proj_w"], np.float32))
    cpb = np.asarray(inputs["c_proj_b"], np.float32)
    mw = np.ascontiguousarray(np.asarray(inputs["mlp_w"], np.float32))
    mb = np.asarray(inputs["mlp_b"], np.float32)

    mw1_bf = mw[:H].astype(NPBF16)
    mw2_bf = mw[H:].astype(NPBF16)
    # gelu bias: mlp_b + c_proj_b @ mlp_w2  (folded host-side)
    beff = (
        mb.astype(np.float64) + cpb.astype(np.float64) @ mw[H:].astype(np.float64)
    ).astype(np.float32)
    beff_t = np.ascontiguousarray(beff.reshape(H // P, P).T)

    in_maps = []
    for c in range(NCORES):
        b, g = c // TP, c % TP
        gs = slice(g * DH, (g + 1) * DH)
        xatdT = np.concatenate([xk[b].T, np.ones((1, S), np.float32)], 0)
        wv = np.concatenate(
            [caw[:, 2 * H + g * DH : 2 * H + (g + 1) * DH],
             cab[None, 2 * H + g * DH : 2 * H + (g + 1) * DH]], 0)
        # [128, 4]: q bias (2 output halves), k bias (2 output halves)
        qkb = np.stack(
            [cab[gs][: P], cab[gs][P:],
             cab[H + g * DH : H + (g + 1) * DH][: P],
             cab[H + g * DH : H + (g + 1) * DH][P:]], 1)
        in_maps.append({
            "xatt": np.ascontiguousarray(xq[b].T),
            "xatd": np.ascontiguousarray(xatdT),
            "xmlp": np.ascontiguousarray(
                xq[b, g * QB : (g + 1) * QB, :].T.astype(NPBF16)),
            "wq": np.ascontiguousarray(caw[:, gs]),
            "wk": np.ascontiguousarray(caw[:, H + g * DH : H + (g + 1) * DH]),
            "wv": np.ascontiguousarray(wv),
            "qkbias": np.ascontiguousarray(qkb),
            "beff": beff_t,
            "maskrep": np.ascontiguousarray(
                np.broadcast_to(mask[b][None, :], (P, S))),
            "cprojw": np.ascontiguousarray(cpw[gs, :]),
            "mlpw1": mw1_bf,
            "mlpw2": mw2_bf,
        })
    return in_maps
